# revision 28
# baseline (speedup 1.0000x reference)
"""AttentionPool3d kernel for 8 Trainium2 NeuronCores.

Shapes (hardcoded): x [8, 512, 8, 16, 16] f32, pos_emb [512, 2049],
w_qkv [1536, 512], b_qkv [1536], w_c [512, 512], b_c [512].
Output: [8, 512] f32.

The reference returns out[:, :, 0] - only attention-query position 0 (the
mean token) is used, so per (batch, head) this is single-query attention:
    scores_h[s] = g_h^T xf[:, s]   with g = sum_{c in h} q0'[c] w_k[c, :]
    p = softmax_s(scores)          (b_k cancels; scores ~ N(0,1) so the
                                    max-subtraction is skipped: exp is safe)
    a0_h = w_v_h (xf @ p_h)        (b_v folds into the output bias row)
    out  = w_c a0 + (w_c b_v + b_c)
Sharding: data-parallel over batch, one batch element per core.

v5: fp16 data path, column-major [128, 4chunk, 512] pieces.  xf = x+pos
via out-of-place DVE adds per block (fast when the engine is warm).
Mean-token chain is latency-trimmed: per-piece row-sum partials race the
DMA (last piece split DVE/Act), q0 into one psum tile + one bias add,
gT computed directly (w_k^T against block-diag q0).  Per-block pipeline:
add -> transposes -> scores -> exp -> PT -> pooled, so only the last
block's work trails the final DMA.  All small transposes are batched
into single psum tiles with one copy out.
"""

import sys

import numpy as np

for p in ("/opt/trn_rl_repo", "/root/.axon_site/_ro/trn_rl_repo"):
    if p not in sys.path:
        sys.path.append(p)

import concourse.bacc as bacc
import concourse.tile as tile
from concourse import mybir
from concourse.bass_utils import run_bass_kernel_spmd
from concourse.masks import make_identity

F32 = mybir.dt.float32
F16 = mybir.dt.float16
AX = mybir.AxisListType
AF = mybir.ActivationFunctionType
ALU = mybir.AluOpType

C = 512          # channels
SD = 2048        # data sequence length (T*H*W)
S = 2049         # + mean token
NCHUNK = 4       # 512 / 128 partition chunks
NB = 4           # 512-column blocks of the data sequence
NH = 8           # heads
CH = 64          # channels per head
NST = 17         # 16 full 128-col s-tiles + mean-token tile (w=1)
SCALE2 = 0.125   # (1/64**0.25)**2 folded into q side (host)

_CACHE = {}


def _build_program():
    nc = bacc.Bacc()

    x_d = nc.declare_dram_parameter("x", [NB, 128, NCHUNK, 512], F16,
                                    isOutput=False)
    pos_d = nc.declare_dram_parameter("pos", [NB, 128, NCHUNK, 512], F16,
                                      isOutput=False)
    wqT_d = nc.declare_dram_parameter("wqT", [128, NCHUNK, C], F16,
                                     isOutput=False)
    wk_d = nc.declare_dram_parameter("wk", [128, NCHUNK, C], F16,
                                    isOutput=False)
    wvT_d = nc.declare_dram_parameter("wvT", [128, NCHUNK, C], F16,
                                     isOutput=False)
    wcT_d = nc.declare_dram_parameter("wcT", [128, NCHUNK, C], F16,
                                     isOutput=False)
    bias_d = nc.declare_dram_parameter("bias", [128, 8], F32, isOutput=False)
    brow_d = nc.declare_dram_parameter("brow", [1, C], F32, isOutput=False)
    out_d = nc.declare_dram_parameter("out", [C], F32, isOutput=True)

    with tile.TileContext(nc) as tc:
        with (
            tc.tile_pool(name="weights", bufs=1) as wpool,
            tc.tile_pool(name="xp", bufs=1) as xpool,
            tc.tile_pool(name="small", bufs=1) as sm,
            tc.tile_pool(name="ptr", bufs=3, space="PSUM") as ptr,
            tc.tile_pool(name="ptr2", bufs=2, space="PSUM") as ptr2,
            tc.tile_pool(name="pmm", bufs=2, space="PSUM") as pmm,
            tc.tile_pool(name="ppol", bufs=1, space="PSUM") as ppol,
        ):
            ident = wpool.tile([128, 128], F16, tag="ident")
            make_identity(nc, ident)
            bias_sb = wpool.tile([128, 8], F32, tag="bias")
            nc.scalar.dma_start(out=bias_sb, in_=bias_d[:, :])
            brow_sb = wpool.tile([1, C], F32, tag="brow")
            nc.scalar.dma_start(out=brow_sb, in_=brow_d[:, :])

            # data pieces on SP: all x first (mean chain), then wq/wk,
            # then pos, then wv/wc - single queue, explicit priority.
            xs, ps_ = [None] * NB, [None] * NB

            def dma_piece(dst_list, src_d, sb, tag):
                t = xpool.tile([128, NCHUNK, 512], F16, tag=f"{tag}{sb}")
                dst_list[sb] = t
                nc.sync.dma_start(out=t, in_=src_d[sb])

            for sb in range(NB):
                dma_piece(xs, x_d, sb, "x")
            wqT_sb = wpool.tile([128, NCHUNK, C], F16, tag="wqT")
            nc.sync.dma_start(
                out=wqT_sb, in_=wqT_d[:, :, :]
            )
            wk_sb = wpool.tile([128, NCHUNK, C], F16, tag="wk")
            nc.sync.dma_start(
                out=wk_sb, in_=wk_d[:, :, :]
            )
            for sb in range(NB):
                dma_piece(ps_, pos_d, sb, "p")
            wvT_sb = wpool.tile([128, NCHUNK, C], F16, tag="wvT")
            nc.sync.dma_start(
                out=wvT_sb, in_=wvT_d[:, :, :]
            )
            wcT_sb = wpool.tile([128, NCHUNK, C], F16, tag="wcT")
            nc.sync.dma_start(
                out=wcT_sb, in_=wcT_d[:, :, :]
            )

            # ---- per-block pipeline ----
            xf = [None] * NB
            xfT = xpool.tile([128, NST, C], F16, tag="xfT")
            e_sb = sm.tile([NH, S], F16, tag="e")
            zparts = sm.tile([NH, 8], F32, tag="zparts")
            PT = sm.tile([128, NST, NH], F16, tag="PT")
            ppool = ppol.tile([NH, C], F32, tag="pool")

            def emit_add(sb):
                t = xpool.tile([128, NCHUNK, 512], F16, tag=f"xf{sb}")
                xf[sb] = t
                nc.vector.tensor_add(t, xs[sb], ps_[sb])

            # ---- mean-token chain, racing the DMA ----
            # per-piece row-sum partials; the last piece is split DVE/Act
            # so its partial costs ~0.6us instead of 2.2.
            psums = sm.tile([128, NCHUNK, NB + 1], F32, tag="psums")
            nc.vector.reduce_sum(psums[:, :, 0:1], xs[0], axis=AX.X)
            for i in range(NCHUNK):
                nc.scalar.activation(xs[1][:, i, :], xs[1][:, i, :],
                                     AF.Copy, accum_out=psums[:, i, 1:2])
            nc.vector.reduce_sum(psums[:, :, 2:3], xs[2], axis=AX.X)
            nc.vector.reduce_sum(psums[:, :, 3:4], xs[3][:, :, 0:256], axis=AX.X)
            for i in range(NCHUNK):
                nc.scalar.activation(xs[3][:, i, 256:512], xs[3][:, i, 256:512],
                                     AF.Copy, accum_out=psums[:, i, 4:5])
            sums = sm.tile([128, NCHUNK], F32, tag="sums")
            nc.vector.reduce_sum(sums, psums, axis=AX.X)
            # xf0 = sums/2048 + pos[:, 0]  (one DVE op)
            xf0_sb = sm.tile([128, NCHUNK], F16, tag="xf0")
            nc.vector.scalar_tensor_tensor(
                out=xf0_sb, in0=sums, scalar=1.0 / SD, in1=bias_sb[:, 4:8],
                op0=ALU.mult, op1=ALU.add,
            )

            # q0 = s^2 (w_q xf0 + b_q): 16 matvecs into one psum tile,
            # one DVE add for the bias
            pq = ptr2.tile([128, NCHUNK], F32, tag="tr2")
            for j in range(NCHUNK):
                for i in range(NCHUNK):
                    nc.tensor.matmul(
                        pq[:, j : j + 1],
                        wqT_sb[:, i, 128 * j : 128 * (j + 1)],
                        xf0_sb[:, i : i + 1],
                        start=(i == 0), stop=(i == NCHUNK - 1),
                    )
            emit_add(0)
            q0_sb = sm.tile([128, NCHUNK], F16, tag="q0")
            nc.vector.tensor_add(q0_sb, pq, bias_sb[:, 0:4])

            # gT[c, h] directly: contract w_k rows against block-diag q0
            qbd = sm.tile([128, NCHUNK, NH], F16, tag="qbd")
            nc.vector.memset(qbd, 0.0)
            for i in range(NCHUNK):
                nc.vector.tensor_copy(qbd[0:CH, i, 2 * i : 2 * i + 1],
                                      q0_sb[0:CH, i : i + 1])
                nc.vector.tensor_copy(qbd[CH:128, i, 2 * i + 1 : 2 * i + 2],
                                      q0_sb[CH:128, i : i + 1])
            emit_add(1)
            pgT = ptr2.tile([128, NCHUNK, NH], F32, tag="tr2")
            for j in range(NCHUNK):
                for i in range(NCHUNK):
                    nc.tensor.matmul(
                        pgT[:, j, :],
                        wk_sb[:, i, 128 * j : 128 * (j + 1)],
                        qbd[:, i, :],
                        start=(i == 0), stop=(i == NCHUNK - 1),
                    )
            gT = sm.tile([128, NCHUNK, NH], F16, tag="gT")
            nc.vector.tensor_copy(gT, pgT)
            emit_add(2)
            emit_add(3)


            def emit_tr(t):
                sb, u = t // 4, t % 4
                pt = ptr.tile([128, NCHUNK, 128], F16, tag="tr")
                for i in range(NCHUNK):
                    nc.tensor.transpose(
                        pt[:, i, :], xf[sb][:, i, 128 * u : 128 * (u + 1)], ident)
                if t % 2 == 0:
                    nc.scalar.copy(xfT[:, t, 0:C], pt)
                else:
                    nc.vector.tensor_copy(xfT[:, t, 0:C], pt)

            def emit_scores(sb):
                psc = pmm.tile([NH, 512], F32, tag="mm")
                for i in range(NCHUNK):
                    nc.tensor.matmul(psc, gT[:, i, :], xf[sb][:, i, :],
                                     start=(i == 0), stop=(i == NCHUNK - 1))
                nc.scalar.activation(
                    e_sb[:, 512 * sb : 512 * (sb + 1)], psc, AF.Exp,
                    accum_out=zparts[:, sb : sb + 1],
                )

            def emit_pt(sb):
                pt = ptr2.tile([128, NCHUNK, NH], F16, tag="tr2")
                for u in range(NCHUNK):
                    t = 4 * sb + u
                    nc.tensor.transpose(pt[:, u, :],
                                        e_sb[:, 128 * t : 128 * (t + 1)],
                                        ident[0:NH, 0:NH])
                nc.vector.tensor_copy(PT[:, 4 * sb : 4 * (sb + 1), :], pt)

            def emit_pooled(sb):
                for u in range(NCHUNK):
                    t = 4 * sb + u
                    nc.tensor.matmul(ppool, PT[:, t, :], xfT[:, t, :],
                                     start=(t == 0), stop=False)

            # mean-token row of xfT (tile 16)
            pt0 = ptr2.tile([1, NCHUNK, 128], F16, tag="tr2")
            for i in range(NCHUNK):
                nc.tensor.transpose(pt0[:, i, :], xf0_sb[:, i : i + 1], ident)
            nc.vector.tensor_copy(xfT[0:1, 16, 0:C], pt0)

            emit_tr(0); emit_tr(1); emit_tr(2); emit_tr(3)
            emit_scores(0)

            # mean-token score column (only needs gT + xf0)
            ps4 = pmm.tile([NH, 1], F32, tag="mm")
            for i in range(NCHUNK):
                nc.tensor.matmul(ps4, gT[:, i, :], xf0_sb[:, i : i + 1],
                                 start=(i == 0), stop=(i == NCHUNK - 1))
            nc.scalar.activation(e_sb[:, SD : SD + 1], ps4, AF.Exp,
                                 accum_out=zparts[:, 4:5])

            emit_tr(4); emit_tr(5); emit_tr(6); emit_tr(7)
            emit_scores(1)
            emit_pt(0)
            emit_pooled(0)

            emit_tr(8); emit_tr(9); emit_tr(10); emit_tr(11)
            emit_scores(2)
            emit_pt(1)
            emit_pooled(1)

            emit_tr(12); emit_tr(13); emit_tr(14); emit_tr(15)
            emit_scores(3)
            emit_pt(2)
            emit_pooled(2)
            emit_pt(3)
            emit_pooled(3)

            # ---- 1/Z ----
            z1 = sm.tile([NH, 1], F32, tag="z1")
            rz = sm.tile([NH, 1], F32, tag="rz")
            nc.vector.reduce_sum(z1, zparts[:, 0:5], axis=AX.X)
            nc.vector.reciprocal(rz, z1)

            # PT tile 16 + last pooled term (mean token, K=1)
            pt16 = ptr2.tile([1, NH], F16, tag="tr2")
            nc.tensor.transpose(pt16, e_sb[:, SD : SD + 1], ident[0:NH, 0:NH])
            PT16 = sm.tile([1, NH], F16, tag="PT16")
            nc.vector.tensor_copy(PT16, pt16)
            nc.tensor.matmul(ppool, PT16, xfT[0:1, 16, :],
                             start=False, stop=True)

            pooled_sb = sm.tile([NH, C], F16, tag="pooled")
            nc.scalar.activation(pooled_sb, ppool, AF.Copy, scale=rz)

            # ---- av[h, c] = (w_v pooled_h)[c] ----
            plT = sm.tile([128, NCHUNK, NH], F16, tag="plT")
            ptl = ptr2.tile([128, NCHUNK, NH], F16, tag="tr2")
            for i in range(NCHUNK):
                nc.tensor.transpose(ptl[:, i, :],
                                    pooled_sb[:, 128 * i : 128 * (i + 1)],
                                    ident[0:NH, 0:NH])
            nc.vector.tensor_copy(plT, ptl)
            pav = pmm.tile([NH, C], F32, tag="mm")
            for i in range(NCHUNK):
                nc.tensor.matmul(pav, plT[:, i, :], wvT_sb[:, i, :],
                                 start=(i == 0), stop=(i == NCHUNK - 1))
            av_sb = sm.tile([NH, C], F16, tag="av")
            nc.vector.tensor_copy(av_sb, pav)

            # ---- a0[c] = av[head(c), c] (b_v folded into brow) ----
            a0_sb = sm.tile([128, NCHUNK], F16, tag="a0")
            pta = ptr2.tile([128, NCHUNK, NH], F16, tag="tr2")
            for i in range(NCHUNK):
                nc.tensor.transpose(pta[:, i, :],
                                    av_sb[:, 128 * i : 128 * (i + 1)],
                                    ident[0:NH, 0:NH])
            for i in range(NCHUNK):
                nc.vector.tensor_copy(a0_sb[0:CH, i : i + 1],
                                      pta[0:CH, i, 2 * i : 2 * i + 1])
                nc.vector.tensor_copy(a0_sb[CH:128, i : i + 1],
                                      pta[CH:128, i, 2 * i + 1 : 2 * i + 2])

            # ---- out = w_c a0 + brow, row form [1, 512] ----
            pout = pmm.tile([1, C], F32, tag="mm")
            for i in range(NCHUNK):
                nc.tensor.matmul(pout, a0_sb[:, i : i + 1], wcT_sb[:, i, :],
                                 start=(i == 0), stop=(i == NCHUNK - 1))
            out_sb = sm.tile([1, C], F32, tag="out")
            nc.vector.tensor_add(out_sb, pout, brow_sb)
            nc.sync.dma_start(out=out_d[:].rearrange("(a c) -> a c", a=1),
                              in_=out_sb)

    nc.compile()
    return nc


def _get_program():
    if "nc" not in _CACHE:
        _CACHE["nc"] = _build_program()
    return _CACHE["nc"]


LAST_RESULT = None


def prepare_in_maps(x, pos_emb, w_qkv, b_qkv, w_c, b_c):
    x = np.asarray(x, dtype=np.float32)
    pos_emb = np.asarray(pos_emb, dtype=np.float32)
    w_qkv = np.asarray(w_qkv, dtype=np.float32)
    b_qkv = np.asarray(b_qkv, dtype=np.float32)
    w_c = np.asarray(w_c, dtype=np.float32)
    b_c = np.asarray(b_c, dtype=np.float32)

    b = x.shape[0]

    def tile_data(a):
        # [512c, 2048s] -> [4sb, 128p, 4i, 512cc]
        return np.ascontiguousarray(
            a.reshape(4, 128, 4, 512).transpose(2, 1, 0, 3))

    def tile_w(a):
        # [512r, 512c] -> [128p, 4i, 512c]
        return np.ascontiguousarray(a.reshape(4, 128, 512).transpose(1, 0, 2))

    xr = np.stack([tile_data(x.reshape(b, C, SD)[i].astype(np.float16))
                   for i in range(b)])
    pos16 = tile_data(pos_emb[:, 1:].astype(np.float16))
    wqT = tile_w((w_qkv[0:C].T * SCALE2).astype(np.float16))
    wk = tile_w(w_qkv[C : 2 * C].astype(np.float16))
    wvT = tile_w(w_qkv[2 * C : 3 * C].T.astype(np.float16))
    wcT = tile_w(w_c.T.astype(np.float16))
    bias = np.zeros((128, 8), np.float32)
    bias[:, 0:4] = (b_qkv[0:C] * SCALE2).reshape(4, 128).T
    bias[:, 4:8] = pos_emb[:, 0].reshape(4, 128).T
    brow = np.ascontiguousarray(
        (w_c @ b_qkv[2 * C : 3 * C] + b_c).reshape(1, C).astype(np.float32))

    shared = {"pos": pos16, "wqT": wqT, "wk": wk, "wvT": wvT, "wcT": wcT,
              "bias": bias, "brow": brow}
    return [dict(shared, x=xr[i]) for i in range(b)]


def kernel(x, pos_emb, w_qkv, b_qkv, w_c, b_c, trace=False):
    global LAST_RESULT
    in_maps = prepare_in_maps(x, pos_emb, w_qkv, b_qkv, w_c, b_c)
    nc = _get_program()
    res = run_bass_kernel_spmd(nc, in_maps, list(range(len(in_maps))), trace=trace)
    LAST_RESULT = res
    return np.stack([res.results[i]["out"] for i in range(len(in_maps))], axis=0)


# revision 29
# speedup vs baseline: 1.0313x; 1.0313x over previous
"""AttentionPool3d kernel for 8 Trainium2 NeuronCores.

Shapes (hardcoded): x [8, 512, 8, 16, 16] f32, pos_emb [512, 2049],
w_qkv [1536, 512], b_qkv [1536], w_c [512, 512], b_c [512].
Output: [8, 512] f32.

The reference returns out[:, :, 0] - only attention-query position 0 (the
mean token) is used, so per (batch, head) this is single-query attention:
    scores_h[s] = g_h^T xf[:, s]   with g = sum_{c in h} q0'[c] w_k[c, :]
    p = softmax_s(scores)          (b_k cancels; scores ~ N(0,1) so the
                                    max-subtraction is skipped: exp is safe)
    a0_h = w_v_h (xf @ p_h)        (b_v folds into the output bias row)
    out  = w_c a0 + (w_c b_v + b_c)
Sharding: data-parallel over batch, one batch element per core.

v5: fp16 data path, column-major [128, 4chunk, 512] pieces.  xf = x+pos
via out-of-place DVE adds per block (fast when the engine is warm).
Mean-token chain is latency-trimmed: per-piece row-sum partials race the
DMA (last piece split DVE/Act), q0 into one psum tile + one bias add,
gT computed directly (w_k^T against block-diag q0).  Per-block pipeline:
add -> transposes -> scores -> exp -> PT -> pooled, so only the last
block's work trails the final DMA.  All small transposes are batched
into single psum tiles with one copy out.
"""

import sys

import numpy as np

for p in ("/opt/trn_rl_repo", "/root/.axon_site/_ro/trn_rl_repo"):
    if p not in sys.path:
        sys.path.append(p)

import concourse.bacc as bacc
import concourse.tile as tile
from concourse import mybir
from concourse.bass_utils import run_bass_kernel_spmd
from concourse.masks import make_identity

F32 = mybir.dt.float32
F16 = mybir.dt.float16
AX = mybir.AxisListType
AF = mybir.ActivationFunctionType
ALU = mybir.AluOpType

C = 512          # channels
SD = 2048        # data sequence length (T*H*W)
S = 2049         # + mean token
NCHUNK = 4       # 512 / 128 partition chunks
NB = 4           # 512-column blocks of the data sequence
NH = 8           # heads
CH = 64          # channels per head
NST = 17         # 16 full 128-col s-tiles + mean-token tile (w=1)
SCALE2 = 0.125   # (1/64**0.25)**2 folded into q side (host)

_CACHE = {}


def _build_program():
    nc = bacc.Bacc()

    x_d = nc.declare_dram_parameter("x", [NB, 128, NCHUNK, 512], F16,
                                    isOutput=False)
    pos_d = nc.declare_dram_parameter("pos", [NB, 128, NCHUNK, 512], F16,
                                      isOutput=False)
    wqT_d = nc.declare_dram_parameter("wqT", [128, NCHUNK, C], F16,
                                     isOutput=False)
    wk_d = nc.declare_dram_parameter("wk", [128, NCHUNK, C], F16,
                                    isOutput=False)
    wvT_d = nc.declare_dram_parameter("wvT", [128, NCHUNK, C], F16,
                                     isOutput=False)
    wcT_d = nc.declare_dram_parameter("wcT", [128, NCHUNK, C], F16,
                                     isOutput=False)
    bias_d = nc.declare_dram_parameter("bias", [128, 8], F32, isOutput=False)
    brow_d = nc.declare_dram_parameter("brow", [1, C], F32, isOutput=False)
    out_d = nc.declare_dram_parameter("out", [C], F32, isOutput=True)

    with tile.TileContext(nc) as tc:
        with (
            tc.tile_pool(name="weights", bufs=1) as wpool,
            tc.tile_pool(name="xp", bufs=1) as xpool,
            tc.tile_pool(name="small", bufs=1) as sm,
            tc.tile_pool(name="ptr", bufs=3, space="PSUM") as ptr,
            tc.tile_pool(name="ptr2", bufs=2, space="PSUM") as ptr2,
            tc.tile_pool(name="pmm", bufs=2, space="PSUM") as pmm,
            tc.tile_pool(name="ppol", bufs=1, space="PSUM") as ppol,
        ):
            ident = wpool.tile([128, 128], F16, tag="ident")
            make_identity(nc, ident)
            bias_sb = wpool.tile([128, 8], F32, tag="bias")
            nc.scalar.dma_start(out=bias_sb, in_=bias_d[:, :])
            brow_sb = wpool.tile([1, C], F32, tag="brow")
            nc.scalar.dma_start(out=brow_sb, in_=brow_d[:, :])

            # data pieces on SP: all x first (mean chain), then wq/wk,
            # then pos, then wv/wc - single queue, explicit priority.
            xs, ps_ = [None] * NB, [None] * NB

            def dma_piece(dst_list, src_d, sb, tag):
                t = xpool.tile([128, NCHUNK, 512], F16, tag=f"{tag}{sb}")
                dst_list[sb] = t
                nc.sync.dma_start(out=t, in_=src_d[sb])

            for sb in range(NB):
                dma_piece(xs, x_d, sb, "x")
            wqT_sb = wpool.tile([128, NCHUNK, C], F16, tag="wqT")
            nc.sync.dma_start(
                out=wqT_sb, in_=wqT_d[:, :, :]
            )
            wk_sb = wpool.tile([128, NCHUNK, C], F16, tag="wk")
            nc.sync.dma_start(
                out=wk_sb, in_=wk_d[:, :, :]
            )
            for sb in range(NB):
                dma_piece(ps_, pos_d, sb, "p")
            wvT_sb = wpool.tile([128, NCHUNK, C], F16, tag="wvT")
            nc.sync.dma_start(
                out=wvT_sb, in_=wvT_d[:, :, :]
            )
            wcT_sb = wpool.tile([128, NCHUNK, C], F16, tag="wcT")
            nc.sync.dma_start(
                out=wcT_sb, in_=wcT_d[:, :, :]
            )

            # ---- mean-token chain, racing the DMA ----
            # per-piece row-sum partials; the last piece is split DVE/Act
            # so its partial costs ~0.6us instead of 2.2.
            psums = sm.tile([128, NCHUNK, NB + 1], F32, tag="psums")
            nc.vector.reduce_sum(psums[:, :, 0:1], xs[0], axis=AX.X)
            for i in range(NCHUNK):
                nc.scalar.activation(xs[1][:, i, :], xs[1][:, i, :],
                                     AF.Copy, accum_out=psums[:, i, 1:2])
            nc.vector.reduce_sum(psums[:, :, 2:3], xs[2], axis=AX.X)
            nc.vector.reduce_sum(psums[:, :, 3:4], xs[3][:, :, 0:256], axis=AX.X)
            for i in range(NCHUNK):
                nc.scalar.activation(xs[3][:, i, 256:512], xs[3][:, i, 256:512],
                                     AF.Copy, accum_out=psums[:, i, 4:5])
            sums = sm.tile([128, NCHUNK], F32, tag="sums")
            nc.vector.reduce_sum(sums, psums, axis=AX.X)
            # xf0 = sums/2048 + pos[:, 0]  (one DVE op)
            xf0_sb = sm.tile([128, NCHUNK], F16, tag="xf0")
            nc.vector.scalar_tensor_tensor(
                out=xf0_sb, in0=sums, scalar=1.0 / SD, in1=bias_sb[:, 4:8],
                op0=ALU.mult, op1=ALU.add,
            )

            # q0 = s^2 (w_q xf0 + b_q): 16 matvecs into one psum tile,
            # one DVE add for the bias
            pq = ptr2.tile([128, NCHUNK], F32, tag="tr2")
            for j in range(NCHUNK):
                for i in range(NCHUNK):
                    nc.tensor.matmul(
                        pq[:, j : j + 1],
                        wqT_sb[:, i, 128 * j : 128 * (j + 1)],
                        xf0_sb[:, i : i + 1],
                        start=(i == 0), stop=(i == NCHUNK - 1),
                    )
            q0_sb = sm.tile([128, NCHUNK], F16, tag="q0")
            nc.vector.tensor_add(q0_sb, pq, bias_sb[:, 0:4])

            # gT[c, h] directly: contract w_k rows against block-diag q0
            qbd = sm.tile([128, NCHUNK, NH], F16, tag="qbd")
            nc.vector.memset(qbd, 0.0)
            for i in range(NCHUNK):
                nc.vector.tensor_copy(qbd[0:CH, i, 2 * i : 2 * i + 1],
                                      q0_sb[0:CH, i : i + 1])
                nc.vector.tensor_copy(qbd[CH:128, i, 2 * i + 1 : 2 * i + 2],
                                      q0_sb[CH:128, i : i + 1])
            pgT = ptr2.tile([128, NCHUNK, NH], F32, tag="tr2")
            for j in range(NCHUNK):
                for i in range(NCHUNK):
                    nc.tensor.matmul(
                        pgT[:, j, :],
                        wk_sb[:, i, 128 * j : 128 * (j + 1)],
                        qbd[:, i, :],
                        start=(i == 0), stop=(i == NCHUNK - 1),
                    )
            gT = sm.tile([128, NCHUNK, NH], F16, tag="gT")
            nc.vector.tensor_copy(gT, pgT)

            # ---- per-block pipeline ----
            xf = [None] * NB
            xfT = xpool.tile([128, NST, C], F16, tag="xfT")
            e_sb = sm.tile([NH, S], F16, tag="e")
            zparts = sm.tile([NH, 8], F32, tag="zparts")
            PT = sm.tile([128, NST, NH], F16, tag="PT")
            ppool = ppol.tile([NH, C], F32, tag="pool")

            def emit_add(sb):
                t = xpool.tile([128, NCHUNK, 512], F16, tag=f"xf{sb}")
                xf[sb] = t
                nc.vector.tensor_add(t, xs[sb], ps_[sb])

            def emit_tr(t):
                sb, u = t // 4, t % 4
                pt = ptr.tile([128, NCHUNK, 128], F16, tag="tr")
                for i in range(NCHUNK):
                    nc.tensor.transpose(
                        pt[:, i, :], xf[sb][:, i, 128 * u : 128 * (u + 1)], ident)
                if t % 2 == 0:
                    nc.scalar.copy(xfT[:, t, 0:C], pt)
                else:
                    nc.vector.tensor_copy(xfT[:, t, 0:C], pt)

            def emit_scores(sb):
                psc = pmm.tile([NH, 512], F32, tag="mm")
                for i in range(NCHUNK):
                    nc.tensor.matmul(psc, gT[:, i, :], xf[sb][:, i, :],
                                     start=(i == 0), stop=(i == NCHUNK - 1))
                nc.scalar.activation(
                    e_sb[:, 512 * sb : 512 * (sb + 1)], psc, AF.Exp,
                    accum_out=zparts[:, sb : sb + 1],
                )

            def emit_pt(sb):
                pt = ptr2.tile([128, NCHUNK, NH], F16, tag="tr2")
                for u in range(NCHUNK):
                    t = 4 * sb + u
                    nc.tensor.transpose(pt[:, u, :],
                                        e_sb[:, 128 * t : 128 * (t + 1)],
                                        ident[0:NH, 0:NH])
                nc.vector.tensor_copy(PT[:, 4 * sb : 4 * (sb + 1), :], pt)

            def emit_pooled(sb):
                for u in range(NCHUNK):
                    t = 4 * sb + u
                    nc.tensor.matmul(ppool, PT[:, t, :], xfT[:, t, :],
                                     start=(t == 0), stop=False)

            # mean-token row of xfT (tile 16)
            pt0 = ptr2.tile([1, NCHUNK, 128], F16, tag="tr2")
            for i in range(NCHUNK):
                nc.tensor.transpose(pt0[:, i, :], xf0_sb[:, i : i + 1], ident)
            nc.vector.tensor_copy(xfT[0:1, 16, 0:C], pt0)

            emit_add(0)
            emit_tr(0); emit_tr(1); emit_tr(2); emit_tr(3)
            emit_scores(0)

            # mean-token score column (only needs gT + xf0)
            ps4 = pmm.tile([NH, 1], F32, tag="mm")
            for i in range(NCHUNK):
                nc.tensor.matmul(ps4, gT[:, i, :], xf0_sb[:, i : i + 1],
                                 start=(i == 0), stop=(i == NCHUNK - 1))
            nc.scalar.activation(e_sb[:, SD : SD + 1], ps4, AF.Exp,
                                 accum_out=zparts[:, 4:5])

            emit_add(1)
            emit_tr(4); emit_tr(5); emit_tr(6); emit_tr(7)
            emit_scores(1)
            emit_pt(0)
            emit_pooled(0)

            emit_add(2)
            emit_tr(8); emit_tr(9); emit_tr(10); emit_tr(11)
            emit_scores(2)
            emit_pt(1)
            emit_pooled(1)

            emit_add(3)
            emit_tr(12); emit_tr(13); emit_tr(14); emit_tr(15)
            emit_scores(3)
            emit_pt(2)
            emit_pooled(2)
            emit_pt(3)
            emit_pooled(3)

            # ---- 1/Z ----
            z1 = sm.tile([NH, 1], F32, tag="z1")
            rz = sm.tile([NH, 1], F32, tag="rz")
            nc.vector.reduce_sum(z1, zparts[:, 0:5], axis=AX.X)
            nc.vector.reciprocal(rz, z1)

            # PT tile 16 + last pooled term (mean token, K=1)
            pt16 = ptr2.tile([1, NH], F16, tag="tr2")
            nc.tensor.transpose(pt16, e_sb[:, SD : SD + 1], ident[0:NH, 0:NH])
            PT16 = sm.tile([1, NH], F16, tag="PT16")
            nc.vector.tensor_copy(PT16, pt16)
            nc.tensor.matmul(ppool, PT16, xfT[0:1, 16, :],
                             start=False, stop=True)

            pooled_sb = sm.tile([NH, C], F16, tag="pooled")
            nc.scalar.activation(pooled_sb, ppool, AF.Copy, scale=rz)

            # ---- av[h, c] = (w_v pooled_h)[c] ----
            plT = sm.tile([128, NCHUNK, NH], F16, tag="plT")
            ptl = ptr2.tile([128, NCHUNK, NH], F16, tag="tr2")
            for i in range(NCHUNK):
                nc.tensor.transpose(ptl[:, i, :],
                                    pooled_sb[:, 128 * i : 128 * (i + 1)],
                                    ident[0:NH, 0:NH])
            nc.vector.tensor_copy(plT, ptl)
            pav = pmm.tile([NH, C], F32, tag="mm")
            for i in range(NCHUNK):
                nc.tensor.matmul(pav, plT[:, i, :], wvT_sb[:, i, :],
                                 start=(i == 0), stop=(i == NCHUNK - 1))
            av_sb = sm.tile([NH, C], F16, tag="av")
            nc.vector.tensor_copy(av_sb, pav)

            # ---- a0[c] = av[head(c), c] (b_v folded into brow) ----
            a0_sb = sm.tile([128, NCHUNK], F16, tag="a0")
            pta = ptr2.tile([128, NCHUNK, NH], F16, tag="tr2")
            for i in range(NCHUNK):
                nc.tensor.transpose(pta[:, i, :],
                                    av_sb[:, 128 * i : 128 * (i + 1)],
                                    ident[0:NH, 0:NH])
            for i in range(NCHUNK):
                nc.vector.tensor_copy(a0_sb[0:CH, i : i + 1],
                                      pta[0:CH, i, 2 * i : 2 * i + 1])
                nc.vector.tensor_copy(a0_sb[CH:128, i : i + 1],
                                      pta[CH:128, i, 2 * i + 1 : 2 * i + 2])

            # ---- out = w_c a0 + brow, row form [1, 512] ----
            pout = pmm.tile([1, C], F32, tag="mm")
            for i in range(NCHUNK):
                nc.tensor.matmul(pout, a0_sb[:, i : i + 1], wcT_sb[:, i, :],
                                 start=(i == 0), stop=(i == NCHUNK - 1))
            out_sb = sm.tile([1, C], F32, tag="out")
            nc.vector.tensor_add(out_sb, pout, brow_sb)
            nc.sync.dma_start(out=out_d[:].rearrange("(a c) -> a c", a=1),
                              in_=out_sb)

    nc.compile()
    return nc


def _get_program():
    if "nc" not in _CACHE:
        _CACHE["nc"] = _build_program()
    return _CACHE["nc"]


LAST_RESULT = None


def prepare_in_maps(x, pos_emb, w_qkv, b_qkv, w_c, b_c):
    x = np.asarray(x, dtype=np.float32)
    pos_emb = np.asarray(pos_emb, dtype=np.float32)
    w_qkv = np.asarray(w_qkv, dtype=np.float32)
    b_qkv = np.asarray(b_qkv, dtype=np.float32)
    w_c = np.asarray(w_c, dtype=np.float32)
    b_c = np.asarray(b_c, dtype=np.float32)

    b = x.shape[0]

    def tile_data(a):
        # [512c, 2048s] -> [4sb, 128p, 4i, 512cc]
        return np.ascontiguousarray(
            a.reshape(4, 128, 4, 512).transpose(2, 1, 0, 3))

    def tile_w(a):
        # [512r, 512c] -> [128p, 4i, 512c]
        return np.ascontiguousarray(a.reshape(4, 128, 512).transpose(1, 0, 2))

    xr = np.stack([tile_data(x.reshape(b, C, SD)[i].astype(np.float16))
                   for i in range(b)])
    pos16 = tile_data(pos_emb[:, 1:].astype(np.float16))
    wqT = tile_w((w_qkv[0:C].T * SCALE2).astype(np.float16))
    wk = tile_w(w_qkv[C : 2 * C].astype(np.float16))
    wvT = tile_w(w_qkv[2 * C : 3 * C].T.astype(np.float16))
    wcT = tile_w(w_c.T.astype(np.float16))
    bias = np.zeros((128, 8), np.float32)
    bias[:, 0:4] = (b_qkv[0:C] * SCALE2).reshape(4, 128).T
    bias[:, 4:8] = pos_emb[:, 0].reshape(4, 128).T
    brow = np.ascontiguousarray(
        (w_c @ b_qkv[2 * C : 3 * C] + b_c).reshape(1, C).astype(np.float32))

    shared = {"pos": pos16, "wqT": wqT, "wk": wk, "wvT": wvT, "wcT": wcT,
              "bias": bias, "brow": brow}
    return [dict(shared, x=xr[i]) for i in range(b)]


def kernel(x, pos_emb, w_qkv, b_qkv, w_c, b_c, trace=False):
    global LAST_RESULT
    in_maps = prepare_in_maps(x, pos_emb, w_qkv, b_qkv, w_c, b_c)
    nc = _get_program()
    res = run_bass_kernel_spmd(nc, in_maps, list(range(len(in_maps))), trace=trace)
    LAST_RESULT = res
    return np.stack([res.results[i]["out"] for i in range(len(in_maps))], axis=0)


# revision 34
# speedup vs baseline: 1.2284x; 1.1911x over previous
"""AttentionPool3d kernel for 8 Trainium2 NeuronCores.

Shapes (hardcoded): x [8, 512, 8, 16, 16] f32, pos_emb [512, 2049],
w_qkv [1536, 512], b_qkv [1536], w_c [512, 512], b_c [512].
Output: [8, 512] f32.

The reference returns out[:, :, 0] - only attention-query position 0 (the
mean token) is used, so per (batch, head) this is single-query attention:
    scores_h[s] = g_h^T xf[:, s]   with g = sum_{c in h} q0'[c] w_k[c, :]
    p = softmax_s(scores)          (b_k cancels; scores ~ N(0,1) so the
                                    max-subtraction is skipped: exp is safe)
    a0_h = w_v_h (xf @ p_h)        (b_v folds into the output bias row)
    out  = w_c a0 + (w_c b_v + b_c)
Sharding: data-parallel over batch, one batch element per core.

v5: fp16 data path, column-major [128, 4chunk, 512] pieces.  xf = x+pos
via out-of-place DVE adds per block (fast when the engine is warm).
Mean-token chain is latency-trimmed: per-piece row-sum partials race the
DMA (last piece split DVE/Act), q0 into one psum tile + one bias add,
gT computed directly (w_k^T against block-diag q0).  Per-block pipeline:
add -> transposes -> scores -> exp -> PT -> pooled, so only the last
block's work trails the final DMA.  All small transposes are batched
into single psum tiles with one copy out.
"""

import sys

import numpy as np

for p in ("/opt/trn_rl_repo", "/root/.axon_site/_ro/trn_rl_repo"):
    if p not in sys.path:
        sys.path.append(p)

import concourse.bacc as bacc
import concourse.tile as tile
from concourse import mybir
from concourse.bass_utils import run_bass_kernel_spmd
from concourse.masks import make_identity

F32 = mybir.dt.float32
F16 = mybir.dt.float16
AX = mybir.AxisListType
AF = mybir.ActivationFunctionType
ALU = mybir.AluOpType

C = 512          # channels
SD = 2048        # data sequence length (T*H*W)
S = 2049         # + mean token
NCHUNK = 4       # 512 / 128 partition chunks
NB = 4           # 512-column blocks of the data sequence
NH = 8           # heads
CH = 64          # channels per head
NST = 17         # 16 full 128-col s-tiles + mean-token tile (w=1)
SCALE2 = 0.125   # (1/64**0.25)**2 folded into q side (host)

_CACHE = {}


def _build_program():
    nc = bacc.Bacc()

    x_d = nc.declare_dram_parameter("x", [NB, 128, NCHUNK, 512], F16,
                                    isOutput=False)
    pos_d = nc.declare_dram_parameter("pos", [NB, 128, NCHUNK, 512], F16,
                                      isOutput=False)
    wqT_d = nc.declare_dram_parameter("wqT", [128, NCHUNK, C], F16,
                                     isOutput=False)
    wk_d = nc.declare_dram_parameter("wk", [128, NCHUNK, C], F16,
                                    isOutput=False)
    wvT_d = nc.declare_dram_parameter("wvT", [128, NCHUNK, C], F16,
                                     isOutput=False)
    wcT_d = nc.declare_dram_parameter("wcT", [128, NCHUNK, C], F16,
                                     isOutput=False)
    bias_d = nc.declare_dram_parameter("bias", [128, 8], F32, isOutput=False)
    brow_d = nc.declare_dram_parameter("brow", [1, C], F32, isOutput=False)
    out_d = nc.declare_dram_parameter("out", [C], F32, isOutput=True)

    with tile.TileContext(nc) as tc:
        with (
            tc.tile_pool(name="weights", bufs=1) as wpool,
            tc.tile_pool(name="xp", bufs=1) as xpool,
            tc.tile_pool(name="small", bufs=1) as sm,
            tc.tile_pool(name="ptr", bufs=3, space="PSUM") as ptr,
            tc.tile_pool(name="ptr2", bufs=2, space="PSUM") as ptr2,
            tc.tile_pool(name="pmm", bufs=2, space="PSUM") as pmm,
            tc.tile_pool(name="ppol", bufs=1, space="PSUM") as ppol,
        ):
            ident = wpool.tile([128, 128], F16, tag="ident")
            make_identity(nc, ident)
            qbd2 = sm.tile([128, NCHUNK, 2], F16, tag="qbd2")
            nc.vector.memset(qbd2, 0.0)
            bias_sb = wpool.tile([128, 8], F32, tag="bias")
            nc.scalar.dma_start(out=bias_sb, in_=bias_d[:, :])
            brow_sb = wpool.tile([1, C], F32, tag="brow")
            nc.scalar.dma_start(out=brow_sb, in_=brow_d[:, :])

            # data pieces on SP: all x first (mean chain), then wq/wk,
            # then pos, then wv/wc - single queue, explicit priority.
            xs, ps_ = [None] * NB, [None] * NB

            def dma_piece(dst_list, src_d, sb, tag):
                t = xpool.tile([128, NCHUNK, 512], F16, tag=f"{tag}{sb}")
                dst_list[sb] = t
                nc.sync.dma_start(out=t, in_=src_d[sb])

            for sb in range(NB):
                dma_piece(xs, x_d, sb, "x")
            wqT_sb = wpool.tile([128, NCHUNK, C], F16, tag="wqT")
            nc.sync.dma_start(
                out=wqT_sb, in_=wqT_d[:, :, :]
            )
            wk_sb = wpool.tile([128, NCHUNK, C], F16, tag="wk")
            nc.sync.dma_start(
                out=wk_sb, in_=wk_d[:, :, :]
            )
            for sb in range(NB):
                dma_piece(ps_, pos_d, sb, "p")
            wvT_sb = wpool.tile([128, NCHUNK, C], F16, tag="wvT")
            nc.sync.dma_start(
                out=wvT_sb, in_=wvT_d[:, :, :]
            )
            wcT_sb = wpool.tile([128, NCHUNK, C], F16, tag="wcT")
            nc.sync.dma_start(
                out=wcT_sb, in_=wcT_d[:, :, :]
            )

            # ---- mean-token chain, racing the DMA ----
            # per-piece row-sum partials; the last piece is split DVE/Act
            # so its partial costs ~0.6us instead of 2.2.
            psums = sm.tile([128, NCHUNK, NB + 1], F32, tag="psums")
            nc.vector.reduce_sum(psums[:, :, 0:1], xs[0], axis=AX.X)
            for i in range(NCHUNK):
                nc.scalar.activation(xs[1][:, i, :], xs[1][:, i, :],
                                     AF.Copy, accum_out=psums[:, i, 1:2])
            nc.vector.reduce_sum(psums[:, :, 2:3], xs[2], axis=AX.X)
            nc.vector.reduce_sum(psums[:, :, 3:4], xs[3][:, :, 0:384], axis=AX.X)
            for i in range(NCHUNK):
                nc.scalar.activation(xs[3][:, i, 384:512], xs[3][:, i, 384:512],
                                     AF.Copy, accum_out=psums[:, i, 4:5])
            sums = sm.tile([128, NCHUNK], F32, tag="sums")
            nc.vector.reduce_sum(sums, psums, axis=AX.X)
            # xf0 = sums/2048 + pos[:, 0]  (one DVE op)
            xf0_sb = sm.tile([128, NCHUNK], F16, tag="xf0")
            nc.vector.scalar_tensor_tensor(
                out=xf0_sb, in0=sums, scalar=1.0 / SD, in1=bias_sb[:, 4:8],
                op0=ALU.mult, op1=ALU.add,
            )

            # q0 = s^2 (w_q xf0 + b_q): 16 matvecs into one psum tile,
            # one DVE add for the bias
            pq = ptr2.tile([128, NCHUNK], F32, tag="tr2")
            for j in range(NCHUNK):
                for i in range(NCHUNK):
                    nc.tensor.matmul(
                        pq[:, j : j + 1],
                        wqT_sb[:, i, 128 * j : 128 * (j + 1)],
                        xf0_sb[:, i : i + 1],
                        start=(i == 0), stop=(i == NCHUNK - 1),
                    )
            q0_sb = sm.tile([128, NCHUNK], F16, tag="q0")
            nc.vector.tensor_add(q0_sb, pq, bias_sb[:, 0:4])

            # gT[c, h] directly: contract w_k rows against block-diag q0
            qbd = sm.tile([128, NCHUNK, NH], F16, tag="qbd")
            nc.vector.memset(qbd, 0.0)
            for i in range(NCHUNK):
                nc.vector.tensor_copy(qbd[0:CH, i, 2 * i : 2 * i + 1],
                                      q0_sb[0:CH, i : i + 1])
                nc.vector.tensor_copy(qbd[CH:128, i, 2 * i + 1 : 2 * i + 2],
                                      q0_sb[CH:128, i : i + 1])
            pgT = ptr2.tile([128, NCHUNK, NH], F32, tag="tr2")
            for j in range(NCHUNK):
                for i in range(NCHUNK):
                    nc.tensor.matmul(
                        pgT[:, j, :],
                        wk_sb[:, i, 128 * j : 128 * (j + 1)],
                        qbd[:, i, :],
                        start=(i == 0), stop=(i == NCHUNK - 1),
                    )
            gT = sm.tile([128, NCHUNK, NH], F16, tag="gT")
            nc.vector.tensor_copy(gT, pgT)

            # ---- per-block pipeline ----
            xf = [None] * NB
            xfT = xpool.tile([128, NST, C], F16, tag="xfT")
            e_sb = sm.tile([NH, S], F16, tag="e")
            zparts = sm.tile([NH, 8], F32, tag="zparts")
            PT = sm.tile([128, NST, NH], F16, tag="PT")
            ppool = ppol.tile([NH, C], F32, tag="pool")

            def emit_add(sb):
                t = xpool.tile([128, NCHUNK, 512], F16, tag=f"xf{sb}")
                xf[sb] = t
                nc.vector.tensor_add(t, xs[sb], ps_[sb])

            def emit_tr(t):
                sb, u = t // 4, t % 4
                pt = ptr.tile([128, NCHUNK, 128], F16, tag="tr")
                for i in range(NCHUNK):
                    nc.tensor.transpose(
                        pt[:, i, :], xf[sb][:, i, 128 * u : 128 * (u + 1)], ident)
                if t % 2 == 0:
                    nc.scalar.copy(xfT[:, t, 0:C], pt)
                else:
                    nc.vector.tensor_copy(xfT[:, t, 0:C], pt)

            def emit_scores(sb):
                psc = pmm.tile([NH, 512], F32, tag="mm")
                for i in range(NCHUNK):
                    nc.tensor.matmul(psc, gT[:, i, :], xf[sb][:, i, :],
                                     start=(i == 0), stop=(i == NCHUNK - 1))
                nc.scalar.activation(
                    e_sb[:, 512 * sb : 512 * (sb + 1)], psc, AF.Exp,
                    accum_out=zparts[:, sb : sb + 1],
                )

            def emit_pt(sb):
                pt = ptr2.tile([128, NCHUNK, NH], F16, tag="tr2")
                for u in range(NCHUNK):
                    t = 4 * sb + u
                    nc.tensor.transpose(pt[:, u, :],
                                        e_sb[:, 128 * t : 128 * (t + 1)],
                                        ident[0:NH, 0:NH])
                nc.vector.tensor_copy(PT[:, 4 * sb : 4 * (sb + 1), :], pt)

            def emit_pooled(sb):
                for u in range(NCHUNK):
                    t = 4 * sb + u
                    nc.tensor.matmul(ppool, PT[:, t, :], xfT[:, t, :],
                                     start=(t == 0), stop=False)

            # mean-token row of xfT (tile 16)
            pt0 = ptr2.tile([1, NCHUNK, 128], F16, tag="tr2")
            for i in range(NCHUNK):
                nc.tensor.transpose(pt0[:, i, :], xf0_sb[:, i : i + 1], ident)
            nc.vector.tensor_copy(xfT[0:1, 16, 0:C], pt0)

            emit_add(0)
            emit_scores(0)
            emit_tr(0); emit_tr(1); emit_tr(2); emit_tr(3)

            # mean-token score column (only needs gT + xf0)
            ps4 = pmm.tile([NH, 1], F32, tag="mm")
            for i in range(NCHUNK):
                nc.tensor.matmul(ps4, gT[:, i, :], xf0_sb[:, i : i + 1],
                                 start=(i == 0), stop=(i == NCHUNK - 1))
            nc.scalar.activation(e_sb[:, SD : SD + 1], ps4, AF.Exp,
                                 accum_out=zparts[:, 4:5])

            emit_add(1)
            emit_scores(1)
            emit_tr(4); emit_tr(5); emit_tr(6); emit_tr(7)
            emit_pt(0)
            emit_pooled(0)

            emit_add(2)
            emit_scores(2)
            emit_tr(8); emit_tr(9); emit_tr(10); emit_tr(11)
            emit_pt(1)
            emit_pooled(1)

            emit_add(3)
            emit_scores(3)
            emit_tr(12); emit_tr(13); emit_tr(14); emit_tr(15)
            emit_pt(2)
            emit_pooled(2)
            emit_pt(3)
            emit_pooled(3)

            # PT tile 16 + last pooled term (mean token, K=1)
            pt16 = ptr2.tile([1, NH], F16, tag="tr2")
            nc.tensor.transpose(pt16, e_sb[:, SD : SD + 1], ident[0:NH, 0:NH])
            PT16 = sm.tile([1, NH], F16, tag="PT16")
            nc.vector.tensor_copy(PT16, pt16)
            nc.tensor.matmul(ppool, PT16, xfT[0:1, 16, :],
                             start=False, stop=True)

            # ---- 1/Z ----
            z1 = sm.tile([NH, 1], F32, tag="z1")
            rz = sm.tile([NH, 1], F32, tag="rz")
            nc.vector.reduce_sum(z1, zparts[:, 0:5], axis=AX.X)
            nc.vector.reciprocal(rz, z1)


            pooled_sb = sm.tile([NH, C], F16, tag="pooled")
            nc.scalar.activation(pooled_sb, ppool, AF.Copy, scale=rz)

            # ---- av[h, c] = (w_v pooled_h)[c] ----
            plT = sm.tile([128, NCHUNK, NH], F16, tag="plT")
            ptl = ptr2.tile([128, NCHUNK, NH], F16, tag="tr2")
            for i in range(NCHUNK):
                nc.tensor.transpose(ptl[:, i, :],
                                    pooled_sb[:, 128 * i : 128 * (i + 1)],
                                    ident[0:NH, 0:NH])
            nc.vector.tensor_copy(plT, ptl)
            # ---- avT[c, h(c)-pair] directly in psum: per c-chunk j only
            #      heads 2j, 2j+1 matter, so the rhs is sliced to those
            #      two plT columns; a0 then falls out as two strided
            #      copies (b_v folded into brow) ----
            pavT = ptr2.tile([128, NCHUNK, 2], F32, tag="tr2")
            for j in range(NCHUNK):
                for i in range(NCHUNK):
                    nc.tensor.matmul(
                        pavT[:, j, :],
                        wvT_sb[:, i, 128 * j : 128 * (j + 1)],
                        plT[:, i, 2 * j : 2 * j + 2],
                        start=(i == 0), stop=(i == NCHUNK - 1),
                    )
            a0_sb = sm.tile([128, NCHUNK], F16, tag="a0")
            nc.vector.tensor_copy(a0_sb[0:CH, :], pavT[0:CH, :, 0:1])
            nc.vector.tensor_copy(a0_sb[CH:128, :], pavT[CH:128, :, 1:2])

            # ---- out = w_c a0 + brow, row form [1, 512] ----
            pout = pmm.tile([1, C], F32, tag="mm")
            for i in range(NCHUNK):
                nc.tensor.matmul(pout, a0_sb[:, i : i + 1], wcT_sb[:, i, :],
                                 start=(i == 0), stop=(i == NCHUNK - 1))
            out_sb = sm.tile([1, C], F32, tag="out")
            nc.vector.tensor_add(out_sb, pout, brow_sb)
            nc.sync.dma_start(out=out_d[:].rearrange("(a c) -> a c", a=1),
                              in_=out_sb)

    nc.compile()
    return nc


def _get_program():
    if "nc" not in _CACHE:
        _CACHE["nc"] = _build_program()
    return _CACHE["nc"]


LAST_RESULT = None


def prepare_in_maps(x, pos_emb, w_qkv, b_qkv, w_c, b_c):
    x = np.asarray(x, dtype=np.float32)
    pos_emb = np.asarray(pos_emb, dtype=np.float32)
    w_qkv = np.asarray(w_qkv, dtype=np.float32)
    b_qkv = np.asarray(b_qkv, dtype=np.float32)
    w_c = np.asarray(w_c, dtype=np.float32)
    b_c = np.asarray(b_c, dtype=np.float32)

    b = x.shape[0]

    def tile_data(a):
        # [512c, 2048s] -> [4sb, 128p, 4i, 512cc]
        return np.ascontiguousarray(
            a.reshape(4, 128, 4, 512).transpose(2, 1, 0, 3))

    def tile_w(a):
        # [512r, 512c] -> [128p, 4i, 512c]
        return np.ascontiguousarray(a.reshape(4, 128, 512).transpose(1, 0, 2))

    xr = np.stack([tile_data(x.reshape(b, C, SD)[i].astype(np.float16))
                   for i in range(b)])
    pos16 = tile_data(pos_emb[:, 1:].astype(np.float16))
    wqT = tile_w((w_qkv[0:C].T * SCALE2).astype(np.float16))
    wk = tile_w(w_qkv[C : 2 * C].astype(np.float16))
    wvT = tile_w(w_qkv[2 * C : 3 * C].T.astype(np.float16))
    wcT = tile_w(w_c.T.astype(np.float16))
    bias = np.zeros((128, 8), np.float32)
    bias[:, 0:4] = (b_qkv[0:C] * SCALE2).reshape(4, 128).T
    bias[:, 4:8] = pos_emb[:, 0].reshape(4, 128).T
    brow = np.ascontiguousarray(
        (w_c @ b_qkv[2 * C : 3 * C] + b_c).reshape(1, C).astype(np.float32))

    shared = {"pos": pos16, "wqT": wqT, "wk": wk, "wvT": wvT, "wcT": wcT,
              "bias": bias, "brow": brow}
    return [dict(shared, x=xr[i]) for i in range(b)]


def kernel(x, pos_emb, w_qkv, b_qkv, w_c, b_c, trace=False):
    global LAST_RESULT
    in_maps = prepare_in_maps(x, pos_emb, w_qkv, b_qkv, w_c, b_c)
    nc = _get_program()
    res = run_bass_kernel_spmd(nc, in_maps, list(range(len(in_maps))), trace=trace)
    LAST_RESULT = res
    return np.stack([res.results[i]["out"] for i in range(len(in_maps))], axis=0)
